# revision 1
# baseline (speedup 1.0000x reference)
"""Trainium2 kernel for nn_BranchModel_9680856285960 (moe_routing).

Math: the reference scatters per-branch sparse weights into dense
(n_br, n_out, n_in) tensors, einsums against x, then takes a context-
gated masked sum over branches followed by relu.  Because the mask-
weighted branch sum commutes with the contraction over input features,
the whole model collapses to a 3-layer dense MLP

    out = relu(relu(x @ Weff1.T) @ Weff2.T) @ W3 + b3

where  Weff_l[o, i] = sum_{r,k} masks_l[ctx, r, o] * w_l[r, o, k]
                                * [idx_l[r, o, k] == i].

The effective-weight fold (a scatter-add over 5.6M index/value pairs) is
data-dependent element-granular addressing, which Trainium2 has no fast
engine for; it is done once on the host here, and the device then runs
the dense pipeline.  Weights/activations stream as fp16 (the kernel is
HBM-bound on the weight stream; fp16 halves it and runs the PE at full
rate with fp32 PSUM accumulation).

Sharding: data-parallel over batch (8 cores x 128 rows), effective
weights replicated per core, activations kept feature-major on chip.
No collectives.
"""

import os
import sys
import numpy as np

for _p in ("/opt/trn_rl_repo",):
    if os.path.isdir(_p) and _p not in sys.path:
        sys.path.append(_p)

from contextlib import ExitStack

from concourse import bass, mybir
import concourse.bacc as bacc
import concourse.tile as tile
from concourse.bass_utils import run_bass_kernel_spmd
from concourse.masks import make_identity

F32 = mybir.dt.float32
F16 = mybir.dt.float16

BATCH, NIN, NH, NOUT = 1024, 784, 2000, 10
NCORES = 8
BS = BATCH // NCORES            # 128 batch rows per core
P = 128


def _tiles(total, step):
    out, o = [], 0
    while o < total:
        out.append((o, min(step, total - o)))
        o += step
    return out


MT1 = _tiles(NIN, P)            # layer-1 contraction tiles: 6x128 + 16
MT2 = _tiles(NH, P)             # layer-2/3 contraction tiles: 15x128 + 80
NCHK = _tiles(NH, 512)          # psum output chunks: 3x512 + 464

# Exposed for the test harness: the BassKernelResults of the last run.
LAST_RESULT = None
_CACHE = {}


def _build_weff(w, idx, mask_row, n_in):
    """Fold masks + branch sum into a dense effective weight matrix.

    Weff[o, i] = sum_{r,k} mask_row[r, o] * w[r, o, k] * [idx[r, o, k] == i]
    """
    n_br, n_out, npb = w.shape
    acc = np.zeros(n_out * n_in, np.float64)
    base = (np.arange(n_out, dtype=np.int64) * n_in)[:, None]
    for r in range(n_br):
        flat = (base + idx[r].astype(np.int64)).ravel()
        vals = (w[r].astype(np.float64) * mask_row[r].astype(np.float64)[:, None]).ravel()
        acc += np.bincount(flat, weights=vals, minlength=n_out * n_in)
    return acc.reshape(n_out, n_in).astype(np.float32)


def _mlp_body(tc, xT, w1t, w2t, w3p, b3r, out):
    nc = tc.nc
    rings = [nc.sync, nc.scalar]          # the two HWDGE rings

    with ExitStack() as ctx:
        const = ctx.enter_context(tc.tile_pool(name="const", bufs=1))
        wp = ctx.enter_context(tc.tile_pool(name="wslab", bufs=1))
        act = ctx.enter_context(tc.tile_pool(name="act", bufs=1))
        pacc = ctx.enter_context(tc.tile_pool(name="pacc", bufs=1, space="PSUM"))
        ptr = ctx.enter_context(tc.tile_pool(name="ptr", bufs=1, space="PSUM"))

        ident = const.tile([P, P], F16, tag="ident")
        make_identity(nc, ident[:])

        # x host-packed as [128, n_tiles, 128] (xp[p, t, b] = xT[t*128+p, b])
        # so the load is one contiguous fast DMA, first on the sync ring --
        # it gates the first layer-1 matmul.
        xbig = const.tile([P, len(MT1), P], F16, tag="xbig")
        nc.sync.dma_start(out=xbig[:], in_=xT)
        xts = [xbig[:sz, t, :] for t, (off, sz) in enumerate(MT1)]

        b3t = const.tile([NOUT, 1], F32, tag="b3")
        nc.gpsimd.dma_start(out=b3t[:], in_=b3r)

        # W3 host-packed as [128 partitions, 16 contraction tiles, 10]
        w3t = const.tile([P, len(MT2), NOUT], F16, tag="w3")
        nc.gpsimd.dma_start(out=w3t[:], in_=w3p)

        # Stream all weight slabs up front (they all fit in SBUF); the two
        # HWDGE rings run in parallel and the PE consumes slabs as they land.
        w1s, w2s = [], []
        for t, (off, sz) in enumerate(MT1):
            slab = wp.tile([sz, NH], F16, name=f"w1s{t}", tag=f"w1s{t}")
            if t < 2:
                # column-split the first slabs so the first matmuls start on
                # the first quarter instead of waiting for the full 512KB
                for noff, nsz in NCHK:
                    rings[t % 2].dma_start(
                        out=slab[:, noff:noff + nsz],
                        in_=w1t[off:off + sz, noff:noff + nsz])
            else:
                rings[t % 2].dma_start(out=slab[:], in_=w1t[off:off + sz, :])
            w1s.append(slab[:])
        for t, (off, sz) in enumerate(MT2):
            # w2 on opposite parity from w1 so the two rings carry equal bytes
            slab = wp.tile([sz, NH], F16, name=f"w2s{t}", tag=f"w2s{t}")
            if t >= len(MT2) - 3 and sz > 64:
                # split the tail slabs so the last arrival quantum is small
                h = sz // 2
                rings[(t + 1) % 2].dma_start(out=slab[:h, :],
                                             in_=w2t[off:off + h, :])
                rings[t % 2].dma_start(out=slab[h:sz, :],
                                       in_=w2t[off + h:off + sz, :])
            else:
                rings[(t + 1) % 2].dma_start(out=slab[:],
                                             in_=w2t[off:off + sz, :])
            w2s.append(slab[:])

        # ---- Layer 1: H1 = relu(x @ Weff1.T), batch on partitions
        h1 = act.tile([P, NH], F16, tag="h1")
        ps1 = [pacc.tile([P, sz], F32, name=f"ps1_{n}", tag=f"ps{n}")
               for n, (_, sz) in enumerate(NCHK)]

        for t in range(len(MT1)):
            for n, (noff, nsz) in enumerate(NCHK):
                nc.tensor.matmul(
                    ps1[n][:],
                    lhsT=xts[t],
                    rhs=w1s[t][:, noff:noff + nsz],
                    start=(t == 0),
                    stop=(t == len(MT1) - 1),
                )
        for n, (noff, nsz) in enumerate(NCHK):
            nc.vector.tensor_scalar_max(h1[:, noff:noff + nsz], ps1[n][:], 0.0)

        # Transpose H1 to feature-major tiles for the layer-2 contraction
        pts = [ptr.tile([P, P], F16, name=f"pt{i}", tag=f"pt{i}")
               for i in range(3)]
        h1Tb = act.tile([P, len(MT2), P], F16, tag="h1Tb")
        h1T = []
        for j, (off, sz) in enumerate(MT2):
            pt = pts[j % 3]
            nc.tensor.transpose(pt[:sz, :], h1[:, off:off + sz], ident[:])
            nc.vector.tensor_copy(h1Tb[:sz, j, :], pt[:sz, :])
            h1T.append(h1Tb[:sz, j, :])

        # ---- Layer 2: H2 = relu(H1 @ Weff2.T)
        h2 = act.tile([P, NH], F16, tag="h2")
        ps2 = [pacc.tile([P, sz], F32, name=f"ps2_{n}", tag=f"ps{n}")
               for n, (_, sz) in enumerate(NCHK)]
        for t in range(len(MT2)):
            for n, (noff, nsz) in enumerate(NCHK):
                nc.tensor.matmul(
                    ps2[n][:],
                    lhsT=h1T[t],
                    rhs=w2s[t][:, noff:noff + nsz],
                    start=(t == 0),
                    stop=(t == len(MT2) - 1),
                )
        # Per-j-tile relu (alternating DVE/ACT) so each transpose can start
        # as soon as its 128 columns are ready — this chain is the kernel tail.
        for j, (off, sz) in enumerate(MT2):
            n = j // 4
            csl = slice(off - NCHK[n][0], off - NCHK[n][0] + sz)
            if j % 2 == 0:
                nc.vector.tensor_scalar_max(h2[:, off:off + sz],
                                            ps2[n][:, csl], 0.0)
            else:
                nc.scalar.activation(h2[:, off:off + sz], ps2[n][:, csl],
                                     mybir.ActivationFunctionType.Relu)

        # Transpose H2 for the layer-3 contraction (copies split DVE/ACT to
        # shorten the end-of-kernel critical path)
        h2Tb = act.tile([P, len(MT2), P], F16, tag="h2Tb")
        h2T = []
        for j, (off, sz) in enumerate(MT2):
            pt = pts[j % 3]
            nc.tensor.transpose(pt[:sz, :], h2[:, off:off + sz], ident[:])
            if j % 4 == 3:
                nc.scalar.copy(h2Tb[:sz, j, :], pt[:sz, :])
            else:
                nc.vector.tensor_copy(h2Tb[:sz, j, :], pt[:sz, :])
            h2T.append(h2Tb[:sz, j, :])

        # ---- Layer 3: outT = W3.T @ H2.T + b3.  Transposed orientation:
        # w3 is the (tiny) stationary operand and the output lands as
        # [10, 128], so the final DRAM write is 10 x 512B descriptors
        # instead of 128 x 40B (the host un-transposes, pure layout).
        ps3 = pacc.tile([NOUT, P], F32, tag="ps3")
        for t, (off, sz) in enumerate(MT2):
            nc.tensor.matmul(
                ps3[:],
                lhsT=w3t[:sz, t, :],
                rhs=h2T[t],
                start=(t == 0),
                stop=(t == len(MT2) - 1),
            )
        o = act.tile([NOUT, P], F32, tag="o")
        nc.vector.tensor_add(o[:], ps3[:], b3t[:].to_broadcast([NOUT, P]))
        nc.sync.dma_start(out=out, in_=o[:])


def _get_program():
    if "nc" in _CACHE:
        return _CACHE["nc"]
    nc = bacc.Bacc("TRN2", target_bir_lowering=False, debug=False,
                   enable_asserts=False, enable_partition_id=False,
                   num_devices=NCORES)
    xT = nc.dram_tensor("xT", [P, len(MT1), BS], F16,
                        kind="ExternalInput").ap()
    w1t = nc.dram_tensor("w1t", [NIN, NH], F16, kind="ExternalInput").ap()
    w2t = nc.dram_tensor("w2t", [NH, NH], F16, kind="ExternalInput").ap()
    w3p = nc.dram_tensor("w3p", [P, len(MT2), NOUT], F16,
                         kind="ExternalInput").ap()
    b3r = nc.dram_tensor("b3r", [NOUT, 1], F32, kind="ExternalInput").ap()
    out = nc.dram_tensor("out", [NOUT, BS], F32, kind="ExternalOutput").ap()
    with tile.TileContext(nc) as tc:
        _mlp_body(tc, xT, w1t, w2t, w3p, b3r, out)
    nc.compile()
    _CACHE["nc"] = nc
    return nc


def kernel(x, w1, idx1, w2, idx2, masks1, masks2, W3, b3, context):
    global LAST_RESULT
    x = np.ascontiguousarray(np.asarray(x, dtype=np.float32))
    ctxi = int(np.asarray(context))

    weff1 = _build_weff(np.asarray(w1), np.asarray(idx1),
                        np.asarray(masks1)[ctxi], NIN)
    weff2 = _build_weff(np.asarray(w2), np.asarray(idx2),
                        np.asarray(masks2)[ctxi], NH)
    w1t = np.ascontiguousarray(weff1.T.astype(np.float16))    # (784, 2000)
    w2t = np.ascontiguousarray(weff2.T.astype(np.float16))    # (2000, 2000)

    # W3 packed to [128, n_tiles, 10]: w3p[m, t, :] = W3[t*128 + m, :]
    w3f = np.asarray(W3).astype(np.float16)
    w3p = np.zeros((P, len(MT2), NOUT), np.float16)
    for t, (off, sz) in enumerate(MT2):
        w3p[:sz, t, :] = w3f[off:off + sz, :]
    b3r = np.ascontiguousarray(
        np.asarray(b3, dtype=np.float32).reshape(NOUT, 1))

    try:
        import antenv.axon_hooks  # noqa: F401
    except Exception:
        os.environ.setdefault("BASS_NEVER_TRACE", "1")

    nc = _get_program()
    in_maps = []
    for c in range(NCORES):
        xs = x[c * BS:(c + 1) * BS].T.astype(np.float16)   # (784, 128)
        xT = np.zeros((P, len(MT1), BS), np.float16)
        for t, (off, sz) in enumerate(MT1):
            xT[:sz, t, :] = xs[off:off + sz, :]
        in_maps.append({"xT": xT, "w1t": w1t, "w2t": w2t, "w3p": w3p,
                        "b3r": b3r})

    LAST_RESULT = run_bass_kernel_spmd(nc, in_maps, list(range(NCORES)))
    return np.concatenate(
        [LAST_RESULT.results[c]["out"].T for c in range(NCORES)], axis=0)



# revision 4
# speedup vs baseline: 1.2758x; 1.2758x over previous
"""Trainium2 kernel for nn_BranchModel_9680856285960 (moe_routing).

Math: the reference scatters per-branch sparse weights into dense
(n_br, n_out, n_in) tensors, einsums against x, then takes a context-
gated masked sum over branches followed by relu.  Because the mask-
weighted branch sum commutes with the contraction over input features,
the whole model collapses to a 3-layer dense MLP

    out = relu(relu(x @ Weff1.T) @ Weff2.T) @ W3 + b3

where  Weff_l[o, i] = sum_{r,k} masks_l[ctx, r, o] * w_l[r, o, k]
                                * [idx_l[r, o, k] == i].

The effective-weight fold (a scatter-add over 5.6M index/value pairs) is
done once on the host; the device runs the dense pipeline.

Two structural wins over a straight dense mapping:
 * ~11% of hidden units have ALL branches masked (0.8^10), so their
   Weff rows are identically zero.  Those units are compacted away on
   the host (2000 -> ~1790, padded to a multiple of 128), shrinking
   both the weight stream and the matmul work.
 * The matmuls run "flipped": the 128x128 weight tile is the stationary
   operand (LDWEIGHTS, FWL-pipelined) and the activation tile is the
   moving operand.  Layer outputs then land feature-major in PSUM, so
   no on-chip transposes are needed anywhere, and layer 3 consumes h2
   directly.

Sharding: data-parallel over batch (8 cores x 128 rows), effective
weights replicated per core.  No collectives.  The kernel is bound by
the fp16 weight stream (~9.7 MB/core); weight chunks are issued on the
two HWDGE rings in exact consumption order so the PE trails the stream
by at most one chunk, and the last chunk is small so the post-stream
tail is ~2 us.
"""

import os
import sys
import numpy as np

for _p in ("/opt/trn_rl_repo",):
    if os.path.isdir(_p) and _p not in sys.path:
        sys.path.append(_p)

from contextlib import ExitStack

from concourse import bass, mybir
import concourse.bacc as bacc
import concourse.tile as tile
from concourse.bass_utils import run_bass_kernel_spmd

F32 = mybir.dt.float32
F16 = mybir.dt.float16

BATCH, NIN, NH, NOUT = 1024, 784, 2000, 10
NCORES = 8
BS = BATCH // NCORES            # 128 batch rows per core
P = 128
K1F, K1R = NIN // P, NIN % P    # 6 full k-tiles of x + 16 ragged rows

# Exposed for the test harness: the BassKernelResults of the last run.
LAST_RESULT = None
_CACHE = {}


def _build_weff(w, idx, mask_row, n_in):
    """Fold masks + branch sum into a dense effective weight matrix.

    Weff[o, i] = sum_{r,k} mask_row[r, o] * w[r, o, k] * [idx[r, o, k] == i]
    """
    n_br, n_out, npb = w.shape
    acc = np.zeros(n_out * n_in, np.float64)
    base = (np.arange(n_out, dtype=np.int64) * n_in)[:, None]
    for r in range(n_br):
        flat = (base + idx[r].astype(np.int64)).ravel()
        vals = (w[r].astype(np.float64) * mask_row[r].astype(np.float64)[:, None]).ravel()
        acc += np.bincount(flat, weights=vals, minlength=n_out * n_in)
    return acc.reshape(n_out, n_in).astype(np.float32)


def _banks(nt):
    """Split nt 128-wide tiles into PSUM banks of up to 4 tiles (512 cols)."""
    return [(q * 4, min(4, nt - q * 4)) for q in range((nt + 3) // 4)]


def _chunks(nt):
    """Weight-stream chunking over i-tiles: pairs, then the last two solo
    (a small final chunk keeps the post-stream tail short)."""
    if nt <= 2:
        return [(t, 1) for t in range(nt)]
    out = []
    t = 0
    while t < nt - 2:
        c = min(2, nt - 2 - t)
        out.append((t, c))
        t += c
    out += [(nt - 2, 1), (nt - 1, 1)]
    return out


def _mlp_body(tc, nt1, nt2, xT, w1a, w1b, w2p, w3p, b3r, out):
    nc = tc.nc
    h1w, h2w = nt1 * P, nt2 * P
    b1, b2 = _banks(nt1), _banks(nt2)

    with ExitStack() as ctx:
        const = ctx.enter_context(tc.tile_pool(name="const", bufs=1))
        wp = ctx.enter_context(tc.tile_pool(name="wslab", bufs=1))
        act = ctx.enter_context(tc.tile_pool(name="act", bufs=1))
        pacc = ctx.enter_context(tc.tile_pool(name="pacc", bufs=1, space="PSUM"))

        # ---- input / small constants
        xbig = const.tile([P, K1F + 1, P], F16, tag="xbig")
        nc.sync.dma_start(out=xbig[:], in_=xT)

        b3t = const.tile([NOUT, 1], F32, tag="b3")
        nc.gpsimd.dma_start(out=b3t[:], in_=b3r)
        w3t = const.tile([P, nt2, NOUT], F16, tag="w3")
        nc.gpsimd.dma_start(out=w3t[:], in_=w3p)

        # ---- weight stream, issued in exact consumption order, chunks
        # alternating between the two HWDGE rings (x + the final two w2
        # chunks pinned to sync so the last chunk rides the wire alone).
        rings = [nc.sync, nc.scalar]
        ring_i = 1

        w1s = []
        for k in range(K1F):
            slab = wp.tile([P, h1w], F16, name=f"w1s{k}", tag=f"w1s{k}")
            rings[ring_i].dma_start(out=slab[:], in_=w1a[:, k, :])
            ring_i ^= 1
            w1s.append(slab[:])
        w1bt = wp.tile([K1R, h1w], F16, tag="w1b")
        rings[ring_i].dma_start(out=w1bt[:], in_=w1b)
        ring_i ^= 1

        w2s = [None] * nt1
        for ci, (t0, cn) in enumerate(_chunks(nt1)):
            slab = wp.tile([P, cn, h2w], F16, name=f"w2s{t0}", tag=f"w2s{t0}")
            ring = nc.sync if t0 >= nt1 - 2 else rings[ring_i]
            ring.dma_start(out=slab[:], in_=w2p[:, t0:t0 + cn, :])
            ring_i ^= 1
            for j in range(cn):
                w2s[t0 + j] = slab[:, j, :]

        # ---- Layer 1 (flipped): h1T[o, b] accumulated per o-tile in PSUM.
        ps1 = [pacc.tile([P, n * P], F32, name=f"ps1_{q}", tag=f"ps{q}")
               for q, (_, n) in enumerate(b1)]
        # PSUM accumulation groups are per 2KB bank (zero region): only the
        # bank's first column-slice opens the group (start zeroes the whole
        # bank), siblings overwrite their still-pending-zero slice, and only
        # the bank's last slice at the final contraction step closes it.
        for k in range(K1F + 1):
            lhs_all = w1s[k] if k < K1F else w1bt[:]
            rhs = xbig[:P, k, :] if k < K1F else xbig[:K1R, k, :]
            for to in range(nt1):
                q, j = to // 4, to % 4
                nc.tensor.matmul(
                    ps1[q][:, j * P:(j + 1) * P],
                    lhsT=lhs_all[:, to * P:(to + 1) * P],
                    rhs=rhs,
                    start=(k == 0 and j == 0),
                    stop=(k == K1F and j == b1[q][1] - 1),
                )
        h1q = [act.tile([P, n * P], F16, name=f"h1q{q}", tag=f"h1q{q}")
               for q, (_, n) in enumerate(b1)]
        for q in range(len(b1)):
            nc.vector.tensor_scalar_max(h1q[q][:], ps1[q][:], 0.0)

        # ---- Layer 2 (flipped): h2T[o, b] per o-tile, contract over i.
        ps2 = [pacc.tile([P, n * P], F32, name=f"ps2_{q}", tag=f"ps{q}")
               for q, (_, n) in enumerate(b2)]
        for t in range(nt1):
            rhs = h1q[t // 4][:, (t % 4) * P:(t % 4 + 1) * P]
            for to in range(nt2):
                q, j = to // 4, to % 4
                nc.tensor.matmul(
                    ps2[q][:, j * P:(j + 1) * P],
                    lhsT=w2s[t][:, to * P:(to + 1) * P],
                    rhs=rhs,
                    start=(t == 0 and j == 0),
                    stop=(t == nt1 - 1 and j == b2[q][1] - 1),
                )
        h2q = [act.tile([P, n * P], F16, name=f"h2q{q}", tag=f"h2q{q}")
               for q, (_, n) in enumerate(b2)]
        for q in range(len(b2)):
            nc.vector.tensor_scalar_max(h2q[q][:], ps2[q][:], 0.0)

        # ---- Layer 3: outT = W3c.T @ h2T + b3, h2 consumed feature-major.
        ps3 = pacc.tile([NOUT, P], F32, tag="ps3")
        for to in range(nt2):
            nc.tensor.matmul(
                ps3[:],
                lhsT=w3t[:, to, :],
                rhs=h2q[to // 4][:, (to % 4) * P:(to % 4 + 1) * P],
                start=(to == 0),
                stop=(to == nt2 - 1),
            )
        o = act.tile([NOUT, P], F32, tag="o")
        nc.vector.tensor_add(o[:], ps3[:], b3t[:].to_broadcast([NOUT, P]))
        nc.sync.dma_start(out=out, in_=o[:])


def _get_program(nt1, nt2):
    key = (nt1, nt2)
    if key in _CACHE:
        return _CACHE[key]
    nc = bacc.Bacc("TRN2", target_bir_lowering=False, debug=False,
                   enable_asserts=False, enable_partition_id=False,
                   num_devices=NCORES)
    xT = nc.dram_tensor("xT", [P, K1F + 1, BS], F16,
                        kind="ExternalInput").ap()
    w1a = nc.dram_tensor("w1a", [P, K1F, nt1 * P], F16,
                         kind="ExternalInput").ap()
    w1b = nc.dram_tensor("w1b", [K1R, nt1 * P], F16,
                         kind="ExternalInput").ap()
    w2p = nc.dram_tensor("w2p", [P, nt1, nt2 * P], F16,
                         kind="ExternalInput").ap()
    w3p = nc.dram_tensor("w3p", [P, nt2, NOUT], F16,
                         kind="ExternalInput").ap()
    b3r = nc.dram_tensor("b3r", [NOUT, 1], F32, kind="ExternalInput").ap()
    out = nc.dram_tensor("out", [NOUT, BS], F32, kind="ExternalOutput").ap()
    with tile.TileContext(nc) as tc:
        _mlp_body(tc, nt1, nt2, xT, w1a, w1b, w2p, w3p, b3r, out)
    nc.compile()
    _CACHE[key] = nc
    return nc


def kernel(x, w1, idx1, w2, idx2, masks1, masks2, W3, b3, context):
    global LAST_RESULT
    x = np.ascontiguousarray(np.asarray(x, dtype=np.float32))
    ctxi = int(np.asarray(context))
    m1 = np.asarray(masks1)[ctxi]
    m2 = np.asarray(masks2)[ctxi]

    weff1 = _build_weff(np.asarray(w1), np.asarray(idx1), m1, NIN)
    weff2 = _build_weff(np.asarray(w2), np.asarray(idx2), m2, NH)

    # Compact away hidden units whose branches are all masked (zero rows).
    j1 = np.flatnonzero((m1 != 0).any(axis=0))
    j2 = np.flatnonzero((m2 != 0).any(axis=0))
    n1, n2 = len(j1), len(j2)
    nt1, nt2 = -(-n1 // P), -(-n2 // P)
    h1w, h2w = nt1 * P, nt2 * P

    w1cT = np.zeros((NIN, h1w), np.float32)
    w1cT[:, :n1] = weff1[j1].T
    w2cT = np.zeros((h1w, h2w), np.float32)
    w2cT[:n1, :n2] = weff2[np.ix_(j2, j1)].T
    w3c = np.zeros((h2w, NOUT), np.float32)
    w3c[:n2] = np.asarray(W3)[j2]

    w1a = np.ascontiguousarray(
        w1cT[:K1F * P].reshape(K1F, P, h1w).transpose(1, 0, 2)).astype(np.float16)
    w1b = np.ascontiguousarray(w1cT[K1F * P:NIN]).astype(np.float16)
    w2p = np.ascontiguousarray(
        w2cT.reshape(nt1, P, h2w).transpose(1, 0, 2)).astype(np.float16)
    w3p = np.ascontiguousarray(
        w3c.reshape(nt2, P, NOUT).transpose(1, 0, 2)).astype(np.float16)
    b3r = np.ascontiguousarray(
        np.asarray(b3, dtype=np.float32).reshape(NOUT, 1))

    try:
        import antenv.axon_hooks  # noqa: F401
    except Exception:
        os.environ.setdefault("BASS_NEVER_TRACE", "1")

    nc = _get_program(nt1, nt2)
    in_maps = []
    for c in range(NCORES):
        xs = x[c * BS:(c + 1) * BS].T.astype(np.float16)   # (784, 128)
        xT = np.zeros((P, K1F + 1, BS), np.float16)
        for k in range(K1F + 1):
            sz = P if k < K1F else K1R
            xT[:sz, k, :] = xs[k * P:k * P + sz, :]
        in_maps.append({"xT": xT, "w1a": w1a, "w1b": w1b, "w2p": w2p,
                        "w3p": w3p, "b3r": b3r})

    LAST_RESULT = run_bass_kernel_spmd(nc, in_maps, list(range(NCORES)))
    return np.concatenate(
        [LAST_RESULT.results[c]["out"].T for c in range(NCORES)], axis=0)


# revision 9
# speedup vs baseline: 1.2930x; 1.0135x over previous
"""Trainium2 kernel for nn_BranchModel_9680856285960 (moe_routing).

Math: the reference scatters per-branch sparse weights into dense
(n_br, n_out, n_in) tensors, einsums against x, then takes a context-
gated masked sum over branches followed by relu.  Because the mask-
weighted branch sum commutes with the contraction over input features,
the whole model collapses to a 3-layer dense MLP

    out = relu(relu(x @ Weff1.T) @ Weff2.T) @ W3 + b3

where  Weff_l[o, i] = sum_{r,k} masks_l[ctx, r, o] * w_l[r, o, k]
                                * [idx_l[r, o, k] == i].

The effective-weight fold (a scatter-add over 5.6M index/value pairs) is
done once on the host; the device runs the dense pipeline.

Two structural wins over a straight dense mapping:
 * ~11% of hidden units have ALL branches masked (0.8^10), so their
   Weff rows are identically zero.  Those units are compacted away on
   the host (2000 -> ~1790, padded to a multiple of 128), shrinking
   both the weight stream and the matmul work.
 * The matmuls run "flipped": the 128x128 weight tile is the stationary
   operand (LDWEIGHTS, FWL-pipelined) and the activation tile is the
   moving operand.  Layer outputs then land feature-major in PSUM, so
   no on-chip transposes are needed anywhere, and layer 3 consumes h2
   directly.

Sharding: data-parallel over batch (8 cores x 128 rows), effective
weights replicated per core.  No collectives.  The kernel is bound by
the fp16 weight stream (~9.7 MB/core); weight chunks are issued on the
two HWDGE rings in exact consumption order so the PE trails the stream
by at most one chunk, and the last chunk is small so the post-stream
tail is ~2 us.
"""

import os
import sys
import numpy as np

for _p in ("/opt/trn_rl_repo",):
    if os.path.isdir(_p) and _p not in sys.path:
        sys.path.append(_p)

from contextlib import ExitStack

from concourse import bass, mybir
import concourse.bacc as bacc
import concourse.tile as tile
from concourse.bass_utils import run_bass_kernel_spmd

F32 = mybir.dt.float32
F16 = mybir.dt.float16

BATCH, NIN, NH, NOUT = 1024, 784, 2000, 10
NCORES = 8
BS = BATCH // NCORES            # 128 batch rows per core
P = 128
K1F, K1R = NIN // P, NIN % P    # 6 full k-tiles of x + 16 ragged rows

# Exposed for the test harness: the BassKernelResults of the last run.
LAST_RESULT = None
_CACHE = {}


def _build_weff(w, idx, mask_row, n_in):
    """Fold masks + branch sum into a dense effective weight matrix.

    Weff[o, i] = sum_{r,k} mask_row[r, o] * w[r, o, k] * [idx[r, o, k] == i]
    """
    n_br, n_out, npb = w.shape
    acc = np.zeros(n_out * n_in, np.float64)
    base = (np.arange(n_out, dtype=np.int64) * n_in)[:, None]
    for r in range(n_br):
        flat = (base + idx[r].astype(np.int64)).ravel()
        vals = (w[r].astype(np.float64) * mask_row[r].astype(np.float64)[:, None]).ravel()
        acc += np.bincount(flat, weights=vals, minlength=n_out * n_in)
    return acc.reshape(n_out, n_in).astype(np.float32)


def _banks(nt):
    """Split nt 128-wide tiles into PSUM banks of up to 4 tiles (512 cols)."""
    return [(q * 4, min(4, nt - q * 4)) for q in range((nt + 3) // 4)]


def _chunks(nt):
    """Weight-stream chunking over i-tiles: pairs, then the last two solo
    (a small final chunk keeps the post-stream tail short)."""
    if nt <= 2:
        return [(t, 1) for t in range(nt)]
    out = []
    t = 0
    while t < nt - 2:
        c = min(2, nt - 2 - t)
        out.append((t, c))
        t += c
    out += [(nt - 2, 1), (nt - 1, 1)]
    return out


def _mlp_body(tc, nt1, nt2, xT, w1a, w1b, w2p, w3p, b3r, out):
    nc = tc.nc
    h1w, h2w = nt1 * P, nt2 * P
    b1, b2 = _banks(nt1), _banks(nt2)

    with ExitStack() as ctx:
        const = ctx.enter_context(tc.tile_pool(name="const", bufs=1))
        wp = ctx.enter_context(tc.tile_pool(name="wslab", bufs=1))
        act = ctx.enter_context(tc.tile_pool(name="act", bufs=1))
        pacc = ctx.enter_context(tc.tile_pool(name="pacc", bufs=1, space="PSUM"))

        # ---- PE warm-up: the HAM clock gate keeps the PE at 1.2 GHz until
        # it sees a ~3.4us fully-busy window.  The real matmuls are paced by
        # the weight stream and never present one, so without this the whole
        # kernel runs at half clock.  Burn ~4us of dummy matmuls (PE is idle
        # waiting on DMA anyway) to flip the gate before layer 1 starts.
        dum = const.tile([P, P], F16, tag="dum")
        nc.vector.memset(dum[:], 0.0)
        psd = pacc.tile([P, P], F32, tag="psdum")
        for _ in range(30):
            nc.tensor.matmul(psd[:], lhsT=dum[:], rhs=dum[:],
                             start=True, stop=True)

        # ---- input / small constants
        xbig = const.tile([P, K1F + 1, P], F16, tag="xbig")
        nc.sync.dma_start(out=xbig[:], in_=xT)

        b3t = const.tile([NOUT, 1], F32, tag="b3")
        nc.gpsimd.dma_start(out=b3t[:], in_=b3r)
        w3t = const.tile([P, nt2, NOUT], F16, tag="w3")
        nc.gpsimd.dma_start(out=w3t[:], in_=w3p)

        # ---- weight stream, issued in exact consumption order, chunks
        # alternating between the two HWDGE rings (x + the final two w2
        # chunks pinned to sync so the last chunk rides the wire alone).
        rings = [nc.sync, nc.scalar]
        ring_i = 0

        w1s = []
        for k in range(K1F):
            slab = wp.tile([P, h1w], F16, name=f"w1s{k}", tag=f"w1s{k}")
            rings[ring_i].dma_start(out=slab[:], in_=w1a[:, k, :])
            ring_i ^= 1
            w1s.append(slab[:])
        w1bt = wp.tile([K1R, h1w], F16, tag="w1b")
        rings[ring_i].dma_start(out=w1bt[:], in_=w1b)
        ring_i ^= 1

        w2s = [None] * nt1
        for ci, (t0, cn) in enumerate(_chunks(nt1)):
            slab = wp.tile([P, cn, h2w], F16, name=f"w2s{t0}", tag=f"w2s{t0}")
            ring = nc.sync if t0 >= nt1 - 2 else rings[ring_i]
            ring.dma_start(out=slab[:], in_=w2p[:, t0:t0 + cn, :])
            ring_i ^= 1
            for j in range(cn):
                w2s[t0 + j] = slab[:, j, :]

        # ---- Layer 1 (flipped): h1T[o, b] accumulated per o-tile in PSUM.
        ps1 = [pacc.tile([P, n * P], F32, name=f"ps1_{q}", tag=f"ps{q}")
               for q, (_, n) in enumerate(b1)]
        # PSUM accumulation groups are per 2KB bank (zero region): only the
        # bank's first column-slice opens the group (start zeroes the whole
        # bank), siblings overwrite their still-pending-zero slice, and only
        # the bank's last slice at the final contraction step closes it.
        h1q = [act.tile([P, n * P], F16, name=f"h1q{q}", tag=f"h1q{q}")
               for q, (_, n) in enumerate(b1)]
        for k in range(K1F):
            for to in range(nt1):
                q, j = to // 4, to % 4
                nc.tensor.matmul(
                    ps1[q][:, j * P:(j + 1) * P],
                    lhsT=w1s[k][:, to * P:(to + 1) * P],
                    rhs=xbig[:P, k, :],
                    start=(k == 0 and j == 0),
                    stop=False,
                )
        # last contraction step bank-by-bank, each bank's relu issued right
        # behind its closing matmuls so layer 2 can start off bank 0 while
        # the PE finishes banks 1..3.
        for q, (t0, n) in enumerate(b1):
            for j in range(n):
                to = t0 + j
                nc.tensor.matmul(
                    ps1[q][:, j * P:(j + 1) * P],
                    lhsT=w1bt[:, to * P:(to + 1) * P],
                    rhs=xbig[:K1R, K1F, :],
                    start=False,
                    stop=(j == n - 1),
                )
            nc.vector.tensor_scalar_max(h1q[q][:], ps1[q][:], 0.0)

        # ---- Layer 2 (flipped): h2T[o, b] per o-tile, contract over i.
        ps2 = [pacc.tile([P, n * P], F32, name=f"ps2_{q}", tag=f"ps{q}")
               for q, (_, n) in enumerate(b2)]
        h2q = [act.tile([P, n * P], F16, name=f"h2q{q}", tag=f"h2q{q}")
               for q, (_, n) in enumerate(b2)]
        for t in range(nt1 - 1):
            rhs = h1q[t // 4][:, (t % 4) * P:(t % 4 + 1) * P]
            for to in range(nt2):
                q, j = to // 4, to % 4
                nc.tensor.matmul(
                    ps2[q][:, j * P:(j + 1) * P],
                    lhsT=w2s[t][:, to * P:(to + 1) * P],
                    rhs=rhs,
                    start=(t == 0 and j == 0),
                    stop=False,
                )
        # Final contraction step bank-by-bank; each bank's relu is split in
        # column halves across DVE and GpSimd so the two run concurrently —
        # this chain (last matmuls -> relu -> layer 3) is the kernel tail.
        t = nt1 - 1
        rhs = h1q[t // 4][:, (t % 4) * P:(t % 4 + 1) * P]
        for q, (t0, n) in enumerate(b2):
            for j in range(n):
                to = t0 + j
                nc.tensor.matmul(
                    ps2[q][:, j * P:(j + 1) * P],
                    lhsT=w2s[t][:, to * P:(to + 1) * P],
                    rhs=rhs,
                    start=False,
                    stop=(j == n - 1),
                )
            h = (n * P) // 2
            nc.vector.tensor_scalar_max(h2q[q][:, :h], ps2[q][:, :h], 0.0)
            nc.scalar.activation(h2q[q][:, h:], ps2[q][:, h:],
                                 mybir.ActivationFunctionType.Relu)

        # ---- Layer 3: outT = W3c.T @ h2T + b3, h2 consumed feature-major.
        ps3 = pacc.tile([NOUT, P], F32, tag="ps3")
        for to in range(nt2):
            nc.tensor.matmul(
                ps3[:],
                lhsT=w3t[:, to, :],
                rhs=h2q[to // 4][:, (to % 4) * P:(to % 4 + 1) * P],
                start=(to == 0),
                stop=(to == nt2 - 1),
            )
        o = act.tile([NOUT, P], F32, tag="o")
        nc.vector.tensor_add(o[:], ps3[:], b3t[:].to_broadcast([NOUT, P]))
        nc.sync.dma_start(out=out, in_=o[:])


def _get_program(nt1, nt2):
    key = (nt1, nt2)
    if key in _CACHE:
        return _CACHE[key]
    nc = bacc.Bacc("TRN2", target_bir_lowering=False, debug=False,
                   enable_asserts=False, enable_partition_id=False,
                   num_devices=NCORES)
    xT = nc.dram_tensor("xT", [P, K1F + 1, BS], F16,
                        kind="ExternalInput").ap()
    w1a = nc.dram_tensor("w1a", [P, K1F, nt1 * P], F16,
                         kind="ExternalInput").ap()
    w1b = nc.dram_tensor("w1b", [K1R, nt1 * P], F16,
                         kind="ExternalInput").ap()
    w2p = nc.dram_tensor("w2p", [P, nt1, nt2 * P], F16,
                         kind="ExternalInput").ap()
    w3p = nc.dram_tensor("w3p", [P, nt2, NOUT], F16,
                         kind="ExternalInput").ap()
    b3r = nc.dram_tensor("b3r", [NOUT, 1], F32, kind="ExternalInput").ap()
    out = nc.dram_tensor("out", [NOUT, BS], F32, kind="ExternalOutput").ap()
    with tile.TileContext(nc) as tc:
        _mlp_body(tc, nt1, nt2, xT, w1a, w1b, w2p, w3p, b3r, out)
    nc.compile()
    _CACHE[key] = nc
    return nc


def kernel(x, w1, idx1, w2, idx2, masks1, masks2, W3, b3, context):
    global LAST_RESULT
    x = np.ascontiguousarray(np.asarray(x, dtype=np.float32))
    ctxi = int(np.asarray(context))
    m1 = np.asarray(masks1)[ctxi]
    m2 = np.asarray(masks2)[ctxi]

    weff1 = _build_weff(np.asarray(w1), np.asarray(idx1), m1, NIN)
    weff2 = _build_weff(np.asarray(w2), np.asarray(idx2), m2, NH)

    # Compact away hidden units whose branches are all masked (zero rows).
    j1 = np.flatnonzero((m1 != 0).any(axis=0))
    j2 = np.flatnonzero((m2 != 0).any(axis=0))
    n1, n2 = len(j1), len(j2)
    nt1, nt2 = -(-n1 // P), -(-n2 // P)
    h1w, h2w = nt1 * P, nt2 * P

    w1cT = np.zeros((NIN, h1w), np.float32)
    w1cT[:, :n1] = weff1[j1].T
    w2cT = np.zeros((h1w, h2w), np.float32)
    w2cT[:n1, :n2] = weff2[np.ix_(j2, j1)].T
    w3c = np.zeros((h2w, NOUT), np.float32)
    w3c[:n2] = np.asarray(W3)[j2]

    w1a = np.ascontiguousarray(
        w1cT[:K1F * P].reshape(K1F, P, h1w).transpose(1, 0, 2)).astype(np.float16)
    w1b = np.ascontiguousarray(w1cT[K1F * P:NIN]).astype(np.float16)
    w2p = np.ascontiguousarray(
        w2cT.reshape(nt1, P, h2w).transpose(1, 0, 2)).astype(np.float16)
    w3p = np.ascontiguousarray(
        w3c.reshape(nt2, P, NOUT).transpose(1, 0, 2)).astype(np.float16)
    b3r = np.ascontiguousarray(
        np.asarray(b3, dtype=np.float32).reshape(NOUT, 1))

    try:
        import antenv.axon_hooks  # noqa: F401
    except Exception:
        os.environ.setdefault("BASS_NEVER_TRACE", "1")

    nc = _get_program(nt1, nt2)
    in_maps = []
    for c in range(NCORES):
        xs = x[c * BS:(c + 1) * BS].T.astype(np.float16)   # (784, 128)
        xT = np.zeros((P, K1F + 1, BS), np.float16)
        for k in range(K1F + 1):
            sz = P if k < K1F else K1R
            xT[:sz, k, :] = xs[k * P:k * P + sz, :]
        in_maps.append({"xT": xT, "w1a": w1a, "w1b": w1b, "w2p": w2p,
                        "w3p": w3p, "b3r": b3r})

    LAST_RESULT = run_bass_kernel_spmd(nc, in_maps, list(range(NCORES)))
    return np.concatenate(
        [LAST_RESULT.results[c]["out"].T for c in range(NCORES)], axis=0)
